# revision 1
# baseline (speedup 1.0000x reference)
"""Multi-head attention Bass kernel for Trainium2, SPMD over 8 NeuronCores.

Problem: B=4, S=2048, D=1024, 16 heads x 64. Sharding: core = (batch b, head-group hg)
with b in 0..3, hg in 0..1 -> each core computes 8 heads of one batch.

Per-core algorithm (all matmuls bf16 operands, fp32 PSUM accumulation):
  - Host supplies X^T (D-major) per batch and per-head-group weight slices, bf16.
  - QKV projections on PE: QT/KT in [2-head-d x 128, seq] layout, V natural [k, d]
    with a ones column appended (denominator fold).
  - Scores computed transposed (S^T[k, q]) via row-tiled pairs of K=64 matmuls
    (two heads concurrently in the PE array).
  - exp on ScalarE directly from PSUM (scale=1/sqrt(dh) folded in), bf16 out.
  - AV: lhsT=[V|1] stationary -> O^T[d(+denom), q] accumulated over k chunks
    (the ones column makes row 64 the softmax denominator for free).
  - Normalize O^T rows by a DMA-broadcast reciprocal of the denominator row;
    store O^T per head; the host gather transposes back to [S, heads*dh].
  - Projections for the next head pair and output normalization are emitted as
    small "filler" closures interleaved into the ScalarE-paced attention loop.
"""
import numpy as np
import ml_dtypes
from contextlib import ExitStack

import concourse.tile as tile
import concourse.mybir as mybir
from concourse import bacc
from concourse.bass_utils import run_bass_kernel_spmd

P = 128
DH = 64
BF = mybir.dt.bfloat16
F32 = mybir.dt.float32


def build_attention(S=2048, D=1024, HPC=8, loop_n=1, ablate=(), pbufs=4, pops=3):
    """Build the per-core SPMD program. HPC = heads per core (even).

    loop_n > 1 wraps the whole body in a hardware loop (for timing)."""
    DC = D // P        # D chunks of 128
    KC = S // P        # k chunks of 128
    NQ = S // 512      # q blocks of 512
    KCP = KC // 2      # kc pairs
    HP = HPC // 2      # head pairs
    CW = HPC * DH      # core output width
    SCALE = 1.0 / float(np.sqrt(DH))

    nc = bacc.Bacc("TRN2")
    xq = nc.dram_tensor("xq", [DC, P, S], BF, kind="ExternalInput")
    xk = nc.dram_tensor("xk", [DC, P, S], BF, kind="ExternalInput")
    xv = nc.dram_tensor("xv", [DC, P, S], BF, kind="ExternalInput")
    wq = nc.dram_tensor("wq", [DC, P, CW], BF, kind="ExternalInput")
    wk = nc.dram_tensor("wk", [DC, P, CW], BF, kind="ExternalInput")
    wv = nc.dram_tensor("wv", [DC, P, CW], BF, kind="ExternalInput")
    out = nc.dram_tensor("out", [HPC, DH, S], F32, kind="ExternalOutput")

    with tile.TileContext(nc) as tc, ExitStack() as ctx:
        xpool = ctx.enter_context(tc.tile_pool(name="x", bufs=1))
        wpool = ctx.enter_context(tc.tile_pool(name="w", bufs=1))
        vpool = ctx.enter_context(tc.tile_pool(name="v", bufs=1))
        qkpool = ctx.enter_context(tc.tile_pool(name="qk", bufs=2))
        ppool = ctx.enter_context(tc.tile_pool(name="p", bufs=pbufs))
        ostag = ctx.enter_context(tc.tile_pool(name="ost", bufs=4))
        outp = ctx.enter_context(tc.tile_pool(name="outp", bufs=4))
        rpool = ctx.enter_context(tc.tile_pool(name="r", bufs=4))
        ps_s = ctx.enter_context(tc.tile_pool(name="ps_s", bufs=1, space="PSUM"))
        ps_o = ctx.enter_context(tc.tile_pool(name="ps_o", bufs=1, space="PSUM"))
        ps_m = ctx.enter_context(tc.tile_pool(name="ps_m", bufs=2, space="PSUM"))
        dpool = ctx.enter_context(tc.tile_pool(name="dscr", bufs=4, space="DRAM"))

        xs, ws = {}, {}
        vt = None

        def emit_loads():
            for name, dram in [("q", wq), ("k", wk), ("v", wv)]:
                t = wpool.tile([P, DC, CW], BF, tag="w" + name, name="w" + name)
                for dc in range(DC):
                    nc.sync.dma_start(t[:, dc, :], dram[dc])
                ws[name] = t
            for name, dram in [("q", xq), ("k", xk), ("v", xv)]:
                t = xpool.tile([P, DC, S], BF, tag="x" + name, name="x" + name)
                for dc in range(DC):
                    nc.sync.dma_start(t[:, dc, :], dram[dc])
                xs[name] = t
            # V for all heads: [p(k in chunk), kc, ch, 0:DH] = V, [..., DH] = 1.0
            nonlocal vt
            vt = vpool.tile([P, KC, HPC, DH + 1], BF, tag="V", name="vt")
            nc.any.memset(vt[:, :, :, DH : DH + 1], 1.0)

        def proj_v_kc(kc):
            pv = ps_m.tile([P, 512], F32, tag="proj", name="pv")[:, :CW]
            for dc in range(DC):
                nc.tensor.matmul(
                    pv,
                    xs["v"][:, dc, kc * P : (kc + 1) * P],
                    ws["v"][:, dc, :],
                    start=(dc == 0),
                    stop=(dc == DC - 1),
                )
            nc.vector.tensor_copy(
                vt[:, kc, :, 0:DH],
                pv.rearrange("p (h d) -> p h d", d=DH),
            )

        def proj_qk_chunk(t, which, hp, qb):
            pp = ps_m.tile([P, 512], F32, tag="proj")
            for dc in range(DC):
                nc.tensor.matmul(
                    pp[:],
                    ws[which][:, dc, hp * P : (hp + 1) * P],
                    xs[which][:, dc, qb * 512 : (qb + 1) * 512],
                    start=(dc == 0),
                    stop=(dc == DC - 1),
                )
            nc.vector.tensor_copy(t[:, qb * 512 : (qb + 1) * 512], pp[:])

        def new_qk(which):
            return qkpool.tile([P, S], BF, tag=which, name=which + "t")

        def proj_qk_fillers(t, which, hp):
            """Projection of one tensor for head pair hp as a list of small
            filler closures (2 accumulating matmuls each + final evacuate)."""
            fillers = []
            for qb in range(NQ):
                state = {}

                def mk(dc0, qb=qb, state=state):
                    def f():
                        if dc0 == 0:
                            state["pp"] = ps_m.tile([P, 512], F32, tag="proj",
                                                    name="pp")
                        pp = state["pp"]
                        for dc in (dc0, dc0 + 1):
                            nc.tensor.matmul(
                                pp[:],
                                ws[which][:, dc, hp * P : (hp + 1) * P],
                                xs[which][:, dc, qb * 512 : (qb + 1) * 512],
                                start=(dc == 0),
                                stop=(dc == DC - 1),
                            )
                        if dc0 == DC - 2:
                            nc.vector.tensor_copy(
                                t[:, qb * 512 : (qb + 1) * 512], pp[:])
                    return f

                fillers += [mk(d) for d in range(0, DC, 2)]
            return fillers

        def finalize_fillers(osbs, hp, qb):
            """Transpose + normalize + store for one finished q block, as
            one closure per (head, 128-row chunk)."""
            fillers = []
            if "nofin" in ablate:
                return []
            for h in (0, 1):
                ch = hp * 2 + h
                osb = osbs[h]
                state = {}

                def rec(osb=osb, state=state):
                    rsb = rpool.tile([1, 512], F32, tag="rc", name="rsb")
                    nc.vector.reciprocal(rsb[:], osb[DH : DH + 1, :])
                    rbc = rpool.tile([DH, 512], F32, tag="rbc", name="rbc")
                    nc.gpsimd.partition_broadcast(rbc[:], rsb[0:1, :])
                    state["rbc"] = rbc

                def norm(ch=ch, qb=qb, osb=osb, state=state):
                    ot = outp.tile([DH, 512], F32, tag="ot", name="ot")
                    nc.vector.tensor_tensor(
                        ot[:], osb[0:DH, :], state["rbc"][:], mybir.AluOpType.mult)
                    nc.sync.dma_start(
                        out[ch, :, qb * 512 : (qb + 1) * 512], ot[:])

                fillers += [rec, norm]
            return fillers

        def attn_block(hp, qb, qt, kt, first, proj_q, fin_q):
            """Attention for head pair hp, q block qb (512 wide)."""
            while len(fin_q) > 2:
                fin_q.pop(0)()
            o_ps = [ps_o.tile([DH + 1, 512], F32, tag=f"O{h}", name=f"O{h}") for h in (0, 1)]
            for kcp in range(KCP):
                s_ps = [ps_s.tile([P, 2, 512], F32, tag=f"S{h}", name=f"S{h}") for h in (0, 1)]
                for j in range(2):
                    kc = 2 * kcp + j
                    # V projection just-in-time during the first attn pass
                    if first and qb == 0 and j == 0:
                        proj_v_kc(2 * kcp)
                        proj_v_kc(2 * kcp + 1)
                    for h in (0, 1):
                        nc.tensor.matmul(
                            s_ps[h][:, j, :],
                            kt[h * DH : (h + 1) * DH, kc * P : (kc + 1) * P],
                            qt[h * DH : (h + 1) * DH, qb * 512 : (qb + 1) * 512],
                            start=True,
                            stop=True,
                        )
                pts = []
                for h in (0, 1):
                    pt = ppool.tile([P, 2, 512], BF, tag="pt")
                    if "noexp" not in ablate:
                        nc.scalar.activation(
                            pt[:], s_ps[h][:], mybir.ActivationFunctionType.Exp, scale=SCALE
                        )
                    else:
                        nc.vector.tensor_copy(pt[:, 0, :16], s_ps[h][:, 0, :16])
                    pts.append(pt)
                if "noav" not in ablate:
                    for h in (0, 1):
                        ch = hp * 2 + h
                        for j in range(2):
                            kc = 2 * kcp + j
                            nc.tensor.matmul(
                                o_ps[h][:],
                                vt[:, kc, ch, :],
                                pts[h][:, j, :],
                                start=(kcp == 0 and j == 0),
                                stop=(kcp == KCP - 1 and j == 1),
                            )
                # interleave deferred work while ScalarE paces the loop
                if not (first and qb == 0) and kcp < KCP - 1:
                    budget = pops
                    while budget and (proj_q or fin_q):
                        (proj_q or fin_q).pop(0)()
                        budget -= 1
            # evacuate O PSUM now; transposes/normalize run as fillers later
            osbs = []
            for h in (0, 1):
                osb = ostag.tile([DH + 1, 512], F32, tag="osb")
                nc.vector.tensor_copy(osb[:], o_ps[h][:])
                osbs.append(osb)
            return osbs

        def emit_body():
            emit_loads()
            qt = new_qk("q")
            kt = new_qk("k")
            for qb in range(NQ):
                proj_qk_chunk(qt, "q", 0, qb)
                proj_qk_chunk(kt, "k", 0, qb)
            proj_q, fin_q = [], []
            for hp in range(HP):
                if hp + 1 < HP and "noproj" not in ablate:
                    qt_next = new_qk("q")
                    kt_next = new_qk("k")
                    proj_q += proj_qk_fillers(qt_next, "q", hp + 1)
                    proj_q += proj_qk_fillers(kt_next, "k", hp + 1)
                elif hp + 1 < HP:
                    qt_next, kt_next = qt, kt
                for qb in range(NQ):
                    osbs = attn_block(hp, qb, qt, kt, first=(hp == 0),
                                      proj_q=proj_q, fin_q=fin_q)
                    fin_q += finalize_fillers(osbs, hp, qb)
                # the next head pair's projections must be fully emitted
                # before its attention reads them
                for f in proj_q:
                    f()
                proj_q = []
                if hp + 1 < HP:
                    qt, kt = qt_next, kt_next
            for f in fin_q:
                f()

        if loop_n > 1:
            with tc.For_i(0, loop_n, 1):
                emit_body()
        else:
            emit_body()

    nc.compile()
    return nc


_NC_CACHE = {}


def _get_nc(S, D, HPC):
    key = (S, D, HPC)
    if key not in _NC_CACHE:
        _NC_CACHE[key] = build_attention(S, D, HPC)
    return _NC_CACHE[key]


def _prep_core_inputs(q_seq, k_seq, v_seq, WQ, WK, WV, b, hg, HPC, D):
    """Host-side shard prep for core (batch b, head group hg)."""
    DC = D // P
    CW = HPC * DH
    bf16 = ml_dtypes.bfloat16

    def xt(x):  # [S, D] -> [DC, P, S] (D-major transpose)
        return np.ascontiguousarray(x.T.reshape(DC, P, -1)).astype(bf16)

    def wslice(w):  # [D, out] -> [DC, P, CW]
        return np.ascontiguousarray(
            w[:, hg * CW : (hg + 1) * CW].reshape(DC, P, CW)
        ).astype(bf16)

    return {
        "xq": xt(q_seq[b]),
        "xk": xt(k_seq[b]),
        "xv": xt(v_seq[b]),
        "wq": wslice(WQ),
        "wk": wslice(WK),
        "wv": wslice(WV),
    }


def kernel(q_seq, k_seq, v_seq, WQ, WK, WV, _trace=False):
    q_seq = np.asarray(q_seq, dtype=np.float32)
    k_seq = np.asarray(k_seq, dtype=np.float32)
    v_seq = np.asarray(v_seq, dtype=np.float32)
    WQ = np.asarray(WQ, dtype=np.float32)
    WK = np.asarray(WK, dtype=np.float32)
    WV = np.asarray(WV, dtype=np.float32)

    B, S, D = q_seq.shape
    NB_HEAD = WQ.shape[1] // DH
    n_cores = 8
    groups_per_batch = n_cores // B          # 2 head groups
    HPC = NB_HEAD // groups_per_batch        # 8 heads per core
    CW = HPC * DH

    nc = _get_nc(S, D, HPC)

    in_maps = []
    for core in range(n_cores):
        b, hg = core // groups_per_batch, core % groups_per_batch
        in_maps.append(_prep_core_inputs(q_seq, k_seq, v_seq, WQ, WK, WV, b, hg, HPC, D))

    res = run_bass_kernel_spmd(
        nc, in_maps, core_ids=list(range(n_cores)), trace=_trace,
        **({"trace_cores": [0], } if _trace else {}),
    )
    if _trace:
        print(f"HW exec time: {res.exec_time_ns} ns")
        if res.instructions_and_trace:
            print("trace:", res.instructions_and_trace[1])

    out = np.empty((B, S, NB_HEAD * DH), dtype=np.float32)
    for core in range(n_cores):
        b, hg = core // groups_per_batch, core % groups_per_batch
        # device output is O^T per head: [HPC, DH, S] -> [S, HPC*DH]
        ot = res.results[core]["out"]
        out[b, :, hg * CW : (hg + 1) * CW] = (
            ot.transpose(2, 0, 1).reshape(S, CW)
        )
    return out



# revision 2
# speedup vs baseline: 2.1384x; 2.1384x over previous
"""Multi-head attention Bass kernel for Trainium2, SPMD over 8 NeuronCores.

Problem: B=4, S=2048, D=1024, 16 heads x 64. Sharding: core = (batch b, head-group hg)
with b in 0..3, hg in 0..1 -> each core computes 8 heads of one batch.

Per-core algorithm (all matmuls bf16 operands, fp32 PSUM accumulation):
  - Host supplies X^T (D-major) per batch and per-head-group weight slices, bf16.
  - QKV projections on PE: QT/KT in [2-head-d x 128, seq] layout, V natural [k, d]
    with a ones column appended (denominator fold).
  - Attention inner loop iterates single kc (128 k positions): the two heads'
    scores matmuls (K=64 contraction each) row-pack into opposite halves of the
    PE array (tile_position auto-derived from base partitions) and run
    concurrently, writing the two banks of one [128, 2(head), 512] PSUM tile.
  - One 1024-col exp on ScalarE covers both heads (scale=1/sqrt(dh) folded in),
    bf16 out. Scores PSUM is double-buffered (2 tiles x 2 banks) and the next
    kc's scores are emitted BEFORE this kc's AV so the PE computes them while
    ScalarE streams exp -> steady state is ScalarE-paced at ~1.05us/kc.
  - AV: lhsT=[V|1] stationary -> O^T[d(+denom), q] accumulated over kc
    (the ones column makes row 64 the softmax denominator for free).
  - Normalize O^T rows by a broadcast reciprocal of the denominator row;
    store O^T per head; the host gather transposes back to [S, heads*dh].
  - Projections for the next head pair and output normalization are emitted as
    small filler closures (single matmul each) interleaved into the
    ScalarE-paced loop.

PSUM budget (8 banks): scores 2x[128,2,512] = 4, O accum 2x[65,512] = 2,
projection staging 2x[128,512] = 2.
"""
import numpy as np
import ml_dtypes
from contextlib import ExitStack

import concourse.tile as tile
import concourse.mybir as mybir
from concourse import bacc
from concourse.bass_utils import run_bass_kernel_spmd

P = 128
DH = 64
BF = mybir.dt.bfloat16
F32 = mybir.dt.float32


def build_attention(S=2048, D=1024, HPC=8, loop_n=1, ablate=(), pbufs=4, pops=2):
    """Build the per-core SPMD program. HPC = heads per core (even).

    loop_n > 1 wraps the whole body in a hardware loop (for timing)."""
    DC = D // P        # D chunks of 128
    KC = S // P        # k chunks of 128
    NQ = S // 512      # q blocks of 512
    HP = HPC // 2      # head pairs
    CW = HPC * DH      # core output width
    SCALE = 1.0 / float(np.sqrt(DH))

    nc = bacc.Bacc("TRN2")
    xq = nc.dram_tensor("xq", [DC, P, S], BF, kind="ExternalInput")
    xk = nc.dram_tensor("xk", [DC, P, S], BF, kind="ExternalInput")
    xv = nc.dram_tensor("xv", [DC, P, S], BF, kind="ExternalInput")
    wq = nc.dram_tensor("wq", [DC, P, CW], BF, kind="ExternalInput")
    wk = nc.dram_tensor("wk", [DC, P, CW], BF, kind="ExternalInput")
    wv = nc.dram_tensor("wv", [DC, P, CW], BF, kind="ExternalInput")
    out = nc.dram_tensor("out", [HPC, DH, S], F32, kind="ExternalOutput")

    with tile.TileContext(nc) as tc, ExitStack() as ctx:
        xpool = ctx.enter_context(tc.tile_pool(name="x", bufs=1))
        wpool = ctx.enter_context(tc.tile_pool(name="w", bufs=1))
        vpool = ctx.enter_context(tc.tile_pool(name="v", bufs=1))
        qkpool = ctx.enter_context(tc.tile_pool(name="qk", bufs=2))
        ppool = ctx.enter_context(tc.tile_pool(name="p", bufs=pbufs))
        ostag = ctx.enter_context(tc.tile_pool(name="ost", bufs=4))
        outp = ctx.enter_context(tc.tile_pool(name="outp", bufs=4))
        rpool = ctx.enter_context(tc.tile_pool(name="r", bufs=4))
        ps_s = ctx.enter_context(tc.tile_pool(name="ps_s", bufs=2, space="PSUM"))
        ps_o = ctx.enter_context(tc.tile_pool(name="ps_o", bufs=1, space="PSUM"))
        ps_m = ctx.enter_context(tc.tile_pool(name="ps_m", bufs=2, space="PSUM"))

        xs, ws = {}, {}
        vt = None

        def emit_loads():
            for name, dram in [("q", wq), ("k", wk), ("v", wv)]:
                t = wpool.tile([P, DC, CW], BF, tag="w" + name, name="w" + name)
                for dc in range(DC):
                    nc.sync.dma_start(t[:, dc, :], dram[dc])
                ws[name] = t
            for name, dram in [("q", xq), ("k", xk), ("v", xv)]:
                t = xpool.tile([P, DC, S], BF, tag="x" + name, name="x" + name)
                for dc in range(DC):
                    nc.sync.dma_start(t[:, dc, :], dram[dc])
                xs[name] = t
            # V for all heads: [p(k in chunk), kc, ch, 0:DH] = V, [..., DH] = 1.0
            nonlocal vt
            vt = vpool.tile([P, KC, HPC, DH + 1], BF, tag="V", name="vt")
            nc.any.memset(vt[:, :, :, DH : DH + 1], 1.0)

        def proj_v_kc(kc):
            pv = ps_m.tile([P, 512], F32, tag="proj", name="pv")[:, :CW]
            for dc in range(DC):
                nc.tensor.matmul(
                    pv,
                    xs["v"][:, dc, kc * P : (kc + 1) * P],
                    ws["v"][:, dc, :],
                    start=(dc == 0),
                    stop=(dc == DC - 1),
                )
            nc.vector.tensor_copy(
                vt[:, kc, :, 0:DH],
                pv.rearrange("p (h d) -> p h d", d=DH),
            )

        def proj_qk_chunk(t, which, hp, qb):
            pp = ps_m.tile([P, 512], F32, tag="proj")
            for dc in range(DC):
                nc.tensor.matmul(
                    pp[:],
                    ws[which][:, dc, hp * P : (hp + 1) * P],
                    xs[which][:, dc, qb * 512 : (qb + 1) * 512],
                    start=(dc == 0),
                    stop=(dc == DC - 1),
                )
            nc.vector.tensor_copy(t[:, qb * 512 : (qb + 1) * 512], pp[:])

        def new_qk(which):
            return qkpool.tile([P, S], BF, tag=which, name=which + "t")

        def proj_qk_fillers(t, which, hp):
            """Projection of one tensor for head pair hp as a list of small
            filler closures (one matmul each; the last also evacuates)."""
            fillers = []
            for qb in range(NQ):
                state = {}

                def mk(dc, qb=qb, state=state):
                    def f():
                        if dc == 0:
                            state["pp"] = ps_m.tile([P, 512], F32, tag="proj",
                                                    name="pp")
                        pp = state["pp"]
                        nc.tensor.matmul(
                            pp[:],
                            ws[which][:, dc, hp * P : (hp + 1) * P],
                            xs[which][:, dc, qb * 512 : (qb + 1) * 512],
                            start=(dc == 0),
                            stop=(dc == DC - 1),
                        )
                        if dc == DC - 1:
                            nc.vector.tensor_copy(
                                t[:, qb * 512 : (qb + 1) * 512], pp[:])
                    return f

                fillers += [mk(d) for d in range(DC)]
            return fillers

        def finalize_fillers(osbs, hp, qb):
            """Transpose + normalize + store for one finished q block, as
            one closure per (head, op)."""
            fillers = []
            if "nofin" in ablate:
                return []
            for h in (0, 1):
                ch = hp * 2 + h
                osb = osbs[h]
                state = {}

                def rec(osb=osb, state=state):
                    rsb = rpool.tile([1, 512], F32, tag="rc", name="rsb")
                    nc.vector.reciprocal(rsb[:], osb[DH : DH + 1, :])
                    rbc = rpool.tile([DH, 512], F32, tag="rbc", name="rbc")
                    nc.gpsimd.partition_broadcast(rbc[:], rsb[0:1, :])
                    state["rbc"] = rbc

                def norm(ch=ch, qb=qb, osb=osb, state=state):
                    ot = outp.tile([DH, 512], F32, tag="ot", name="ot")
                    nc.vector.tensor_tensor(
                        ot[:], osb[0:DH, :], state["rbc"][:], mybir.AluOpType.mult)
                    nc.sync.dma_start(
                        out[ch, :, qb * 512 : (qb + 1) * 512], ot[:])

                fillers += [rec, norm]
            return fillers

        def attn_block(hp, qb, qt, kt, first, proj_q, fin_q):
            """Attention for head pair hp, q block qb (512 wide)."""
            while len(fin_q) > 2:
                fin_q.pop(0)()
            o_ps = [ps_o.tile([DH + 1, 512], F32, tag=f"O{h}", name=f"O{h}")
                    for h in (0, 1)]

            def emit_scores(kc):
                s = ps_s.tile([P, 2, 512], F32, tag="S", name="s")
                for h in (0, 1):
                    # K=64 contraction; the two heads' lhsT/rhs live on
                    # opposite partition halves -> row-packed in the PE array
                    nc.tensor.matmul(
                        s[:, h, :],
                        kt[h * DH : (h + 1) * DH, kc * P : (kc + 1) * P],
                        qt[h * DH : (h + 1) * DH, qb * 512 : (qb + 1) * 512],
                        start=True,
                        stop=True,
                    )
                return s

            s_cur = emit_scores(0)
            for kc in range(KC):
                pt = ppool.tile([P, 2, 512], BF, tag="pt")
                if "noexp" not in ablate:
                    nc.scalar.activation(
                        pt[:], s_cur[:], mybir.ActivationFunctionType.Exp,
                        scale=SCALE)
                else:
                    nc.vector.tensor_copy(pt[:, 0, :16], s_cur[:, 0, :16])
                if kc + 1 < KC:
                    s_cur = emit_scores(kc + 1)
                # V projection just-in-time during the first attn pass
                if first and qb == 0:
                    proj_v_kc(kc)
                if "noav" not in ablate:
                    for h in (0, 1):
                        ch = hp * 2 + h
                        nc.tensor.matmul(
                            o_ps[h][:],
                            vt[:, kc, ch, :],
                            pt[:, h, :],
                            start=(kc == 0),
                            stop=(kc == KC - 1),
                        )
                # interleave deferred work while ScalarE paces the loop
                if not (first and qb == 0) and kc < KC - 1:
                    budget = pops
                    while budget and (proj_q or fin_q):
                        (proj_q or fin_q).pop(0)()
                        budget -= 1
            # evacuate O PSUM now; normalize runs as fillers later
            osbs = []
            for h in (0, 1):
                osb = ostag.tile([DH + 1, 512], F32, tag="osb")
                nc.vector.tensor_copy(osb[:], o_ps[h][:])
                osbs.append(osb)
            return osbs

        def emit_body():
            emit_loads()
            qt = new_qk("q")
            kt = new_qk("k")
            for qb in range(NQ):
                proj_qk_chunk(qt, "q", 0, qb)
                proj_qk_chunk(kt, "k", 0, qb)
            proj_q, fin_q = [], []
            for hp in range(HP):
                if hp + 1 < HP and "noproj" not in ablate:
                    qt_next = new_qk("q")
                    kt_next = new_qk("k")
                    proj_q += proj_qk_fillers(qt_next, "q", hp + 1)
                    proj_q += proj_qk_fillers(kt_next, "k", hp + 1)
                elif hp + 1 < HP:
                    qt_next, kt_next = qt, kt
                for qb in range(NQ):
                    osbs = attn_block(hp, qb, qt, kt, first=(hp == 0),
                                      proj_q=proj_q, fin_q=fin_q)
                    fin_q += finalize_fillers(osbs, hp, qb)
                # the next head pair's projections must be fully emitted
                # before its attention reads them
                for f in proj_q:
                    f()
                proj_q = []
                if hp + 1 < HP:
                    qt, kt = qt_next, kt_next
            for f in fin_q:
                f()

        if loop_n > 1:
            with tc.For_i(0, loop_n, 1):
                emit_body()
        else:
            emit_body()

    nc.compile()
    return nc


_NC_CACHE = {}


def _get_nc(S, D, HPC):
    key = (S, D, HPC)
    if key not in _NC_CACHE:
        _NC_CACHE[key] = build_attention(S, D, HPC)
    return _NC_CACHE[key]


def _prep_core_inputs(q_seq, k_seq, v_seq, WQ, WK, WV, b, hg, HPC, D):
    """Host-side shard prep for core (batch b, head group hg)."""
    DC = D // P
    CW = HPC * DH
    bf16 = ml_dtypes.bfloat16

    def xt(x):  # [S, D] -> [DC, P, S] (D-major transpose)
        return np.ascontiguousarray(x.T.reshape(DC, P, -1)).astype(bf16)

    def wslice(w):  # [D, out] -> [DC, P, CW]
        return np.ascontiguousarray(
            w[:, hg * CW : (hg + 1) * CW].reshape(DC, P, CW)
        ).astype(bf16)

    return {
        "xq": xt(q_seq[b]),
        "xk": xt(k_seq[b]),
        "xv": xt(v_seq[b]),
        "wq": wslice(WQ),
        "wk": wslice(WK),
        "wv": wslice(WV),
    }


def kernel(q_seq, k_seq, v_seq, WQ, WK, WV, _trace=False):
    q_seq = np.asarray(q_seq, dtype=np.float32)
    k_seq = np.asarray(k_seq, dtype=np.float32)
    v_seq = np.asarray(v_seq, dtype=np.float32)
    WQ = np.asarray(WQ, dtype=np.float32)
    WK = np.asarray(WK, dtype=np.float32)
    WV = np.asarray(WV, dtype=np.float32)

    B, S, D = q_seq.shape
    NB_HEAD = WQ.shape[1] // DH
    n_cores = 8
    groups_per_batch = n_cores // B          # 2 head groups
    HPC = NB_HEAD // groups_per_batch        # 8 heads per core
    CW = HPC * DH

    nc = _get_nc(S, D, HPC)

    in_maps = []
    for core in range(n_cores):
        b, hg = core // groups_per_batch, core % groups_per_batch
        in_maps.append(_prep_core_inputs(q_seq, k_seq, v_seq, WQ, WK, WV, b, hg, HPC, D))

    res = run_bass_kernel_spmd(
        nc, in_maps, core_ids=list(range(n_cores)), trace=_trace,
        **({"trace_cores": [0], } if _trace else {}),
    )
    if _trace:
        print(f"HW exec time: {res.exec_time_ns} ns")
        if res.instructions_and_trace:
            print("trace:", res.instructions_and_trace[1])

    out = np.empty((B, S, NB_HEAD * DH), dtype=np.float32)
    for core in range(n_cores):
        b, hg = core // groups_per_batch, core % groups_per_batch
        # device output is O^T per head: [HPC, DH, S] -> [S, HPC*DH]
        ot = res.results[core]["out"]
        out[b, :, hg * CW : (hg + 1) * CW] = (
            ot.transpose(2, 0, 1).reshape(S, CW)
        )
    return out
